# revision 15
# baseline (speedup 1.0000x reference)
"""Trainium2 Bass kernel for GCN ExitBlock: out = (adj @ (x @ gc_W) + gc_b) @ fc_W + fc_b.

Strategy (8 NeuronCores, SPMD, no collectives):
  - Reassociate: out = ((adj @ x) @ gc_W + gc_b) @ fc_W + fc_b, row-sharding the
    output so core c computes rows [1500c, 1500(c+1)).
  - The kernel is HBM-bound on streaming adj (576 MB fp32).  Quantize it to
    fp8 e4m3 with a per-row zero point: adj[i,:] = mu_i + D[i,:], where D is
    quantized (uniform residual in [-1/N, 1/N] uses the fp8 grid ~2x better
    than the one-sided raw values).  HBM traffic drops 4x -> ~19 MB/core.
  - The rank-1 zero-point term mu_i * colsum(x) @ W2 is reconstructed exactly
    on-device: an all-ones extra moving column makes the main matmul emit
    colsum(x_q) for free, and a 1-partition outer-product matmul adds
    cs2 (x) muT into the output PSUM group.
  - x is split into fp8 (hi, lo) column pairs (64 stationary columns) so x's
    quantization error is second-order; scales are folded into the fused
    classifier weights W2 = gc_W @ fc_W on device via a per-partition gamma.
  - Main loop: DoubleRow fp8 matmuls contract 256 k-rows per pass
    (stationary [128,2,64] = x pairs, moving [128,2,1504] = D^T pairs).
    k pair-tiles are batched into ~1.5 MB slabs on 2 alternating DMA rings.
  - Epilogue fused on-device: outT = W2aug^T @ g + cs2 (x) muT + c per column
    chunk; host gathers the 8 outT blocks ([16, 1500]) into [12000, 16].

Measured-precision note: full-pipeline numpy sim gives rel err 1.24e-2 vs the
fp32 reference (gate 2e-2); the error is dominated by e4m3 quantization of D.
HBM-bound: ~19 MB/core @ ~367 GB/s => ~52 us roofline.
"""
import sys

sys.path.insert(0, "/opt/trn_rl_repo")

import numpy as np
import ml_dtypes

F8 = ml_dtypes.float8_e4m3

N, NHID, NCLASS, NCORES = 12000, 32, 16, 8
R = N // NCORES            # 1500 rows per core
RP = 1504                  # padded moving columns; col 1500 = ones (colsum), rest 0
KP = 128                   # partitions per sub-tile
NT2 = 47                   # pair-tiles (12032 padded k rows / 256)
NPAD = NT2 * 2 * KP        # 12032
NH2 = 2 * NHID             # 64 stationary cols: [x_hi | x_lo]
# pair-tiles per slab (5 => ~1.9 MB).  Every dma_start costs ~0.6 us of ring
# dead time, so fewer/bigger slabs win (small-slab tapers measured strictly
# worse); taper ONLY the tail so the rings' final slabs don't land a big
# matmul backlog after the last byte.  Alternating rings gives 24/23
# pair-tiles per ring.  A START taper measured worse (PE start latency is
# irrelevant -- the PE always catches up mid-stream).
GROUPS = [5] * 8 + [3, 2, 1, 1]
GMAX = max(GROUPS)
assert sum(GROUPS) == NT2
R_SPLITS = [(0, 512), (512, 512), (1024, RP - 1024)]          # psum bank <= 512

_cached = {}


def _enable_ldw_opt():
    """Flip walrus --enable-ldw-opt to true for this kernel's compile.

    concourse hardcodes it off; enabling lets walrus dedupe the three
    identical back-to-back LDWEIGHTS per pair-tile (same stationary operand
    for the three output-column chunks), removing a PE stall before nearly
    every matmul.
    """
    if _cached.get("ldw_patched"):
        return
    from concourse import bass_utils

    orig = bass_utils.run_command

    def patched(cmd, *a, **kw):
        if isinstance(cmd, list):
            cmd = ["--enable-ldw-opt=true" if c == "--enable-ldw-opt=false" else c
                   for c in cmd]
        return orig(cmd, *a, **kw)

    bass_utils.run_command = patched
    _cached["ldw_patched"] = True


def _build_nc():
    import concourse.bacc as bacc
    import concourse.mybir as mybir
    from concourse import tile

    f32 = mybir.dt.float32
    f32r = mybir.dt.float32r
    f8 = mybir.dt.float8e4
    DR = mybir.MatmulPerfMode.DoubleRow

    nc = bacc.Bacc()
    xP_d = nc.declare_dram_parameter("xP", [KP, NT2 * 2 * NH2], f8, isOutput=False)
    adjT_d = nc.declare_dram_parameter("adjT", [NT2 * KP, 2 * RP], f8, isOutput=False)
    gcWT2_d = nc.declare_dram_parameter("gcWT2", [NHID, NH2], f32, isOutput=False)
    fcW_d = nc.declare_dram_parameter("fcW", [NHID, NCLASS], f32, isOutput=False)
    # cvec = [gc_b; 1] followed by [fc_W; fc_b] rows -> bias vector via matmul
    fcWb_d = nc.declare_dram_parameter("fcWb", [NHID + 1, NCLASS], f32, isOutput=False)
    gcb1_d = nc.declare_dram_parameter("gcb1", [NHID + 1, 1], f32, isOutput=False)
    gam_d = nc.declare_dram_parameter("gam", [NH2, 1], f32, isOutput=False)
    muT_d = nc.declare_dram_parameter("muT", [1, RP], f32r, isOutput=False)
    outT_d = nc.declare_dram_parameter("outT", [NCLASS, R], f32, isOutput=True)

    with tile.TileContext(nc) as tc:
        with (
            tc.tile_pool(name="cst", bufs=1) as cst,
            tc.tile_pool(name="adj", bufs=6) as adjp,
            tc.tile_pool(name="ps_g", bufs=1, space="PSUM") as ps_g,
            tc.tile_pool(name="ps_e", bufs=1, space="PSUM") as ps_e,
        ):
            # ---- constant preloads (small weights on the scalar ring) ----
            x_sb = cst.tile([KP, NT2, 2, NH2], f8)
            gcWT2_sb = cst.tile([NHID, NH2], f32)
            nc.scalar.dma_start(gcWT2_sb[:], gcWT2_d[:])
            fcW_sb = cst.tile([NHID, NCLASS], f32)
            nc.scalar.dma_start(fcW_sb[:], fcW_d[:])
            fcWb_sb = cst.tile([NHID + 1, NCLASS], f32)
            nc.scalar.dma_start(fcWb_sb[:], fcWb_d[:])
            gcb1_sb = cst.tile([NHID + 1, 1], f32)
            nc.scalar.dma_start(gcb1_sb[:], gcb1_d[:])
            gam_sb = cst.tile([NH2, 1], f32)
            nc.scalar.dma_start(gam_sb[:], gam_d[:])
            mu_sb = cst.tile([1, RP], f32r)
            nc.sync.dma_start(mu_sb[:], muT_d[:])

            gps = [ps_g.tile([NH2, n], f32, name=f"gps{j}", tag=f"gps{j}")
                   for j, (_, n) in enumerate(R_SPLITS)]

            # bias vector c = fcWb.T @ [gc_b; 1] = fc_W.T gc_b + fc_b  [16, 1]
            c_ps = ps_e.tile([NCLASS, 1], f32, name="c_ps", tag="c_ps")
            nc.tensor.matmul(c_ps[:], fcWb_sb[:], gcb1_sb[:], start=True, stop=True)
            c_sb = cst.tile([NCLASS, 1], f32)
            nc.vector.tensor_copy(c_sb[:], c_ps[:])
            # stacked fused classifier weights [W2; W2] scaled by gamma  [64, 16]
            w2_ps = ps_e.tile([NH2, NCLASS], f32, name="w2_ps", tag="w2_ps")
            nc.tensor.matmul(w2_ps[:], gcWT2_sb[:], fcW_sb[:], start=True, stop=True)
            W2aug_sb = cst.tile([NH2, NCLASS], f32r)
            nc.vector.tensor_scalar_mul(W2aug_sb[:], w2_ps[:], gam_sb[:, 0:1])

            # ---- main streaming loop: g[64,1504] += xpair.T @(DR) DTpair ----
            # All of x is loaded upfront (one transfer per ring) -- it lives
            # in SBUF for the whole kernel, and per-slab x chunks just added
            # ~0.6 us of ring dead time each.
            xP4 = xP_d.rearrange("p (t i m) -> p t i m", i=2, m=NH2)
            nc.sync.dma_start(x_sb[:, 0:24, :, :], xP4[:, 0:24, :, :])
            nc.scalar.dma_start(x_sb[:, 24:NT2, :, :], xP4[:, 24:NT2, :, :])
            s = 0          # global pair-tile index
            k0 = 0
            # 2 HWDGE rings (sync/scalar).  A third ring on gpsimd (SWDGE)
            # measured WORSE: all queues sank to ~100 GB/s (aggregate 196 vs
            # 373 GB/s with two HWDGE rings, which already sit at the ~358
            # GB/s per-NC HBM limit).
            for g, G in enumerate(GROUPS):
                eng = nc.sync if (g % 2 == 0) else nc.scalar
                a_sb = adjp.tile([KP, GMAX, 2, RP], f8, name="a_sb", tag="a")
                eng.dma_start(
                    a_sb[:, :G, :, :],
                    adjT_d[k0:k0 + KP * G, :].rearrange(
                        "(p j) (i r) -> p j i r", j=G, i=2))
                for j in range(G):
                    st = (s == 0)
                    sp = (s == NT2 - 1)
                    for q, (c0, cn) in enumerate(R_SPLITS):
                        nc.tensor.matmul(gps[q][:, :cn], x_sb[:, s, :, :],
                                         a_sb[:, j, :, c0:c0 + cn],
                                         start=st, stop=sp, perf_mode=DR)
                    s += 1
                k0 += KP * G

            # ---- epilogue: outT = W2aug.T @ g + cs2 (x) muT + c ----
            # Parallelized across engines: the colsum column is copied first
            # (it feeds cs2), the three big PSUM->SBUF copies run on
            # vector/scalar concurrently, each chunk gets its own PSUM bank,
            # and the bias adds alternate vector/scalar.
            g2_sb = cst.tile([NH2, RP], f32r)
            o_sb = cst.tile([NCLASS, RP], f32)
            nc.vector.tensor_copy(g2_sb[:, R:R + 1], gps[2][:, R - 1024:R - 1023])
            # cs2 = colsum_q @ (gamma*W2stack)  [1, 16] on partition 0
            cs2_ps = ps_e.tile([1, NCLASS], f32, name="cs2_ps", tag="cs2_ps")
            nc.tensor.matmul(cs2_ps[:], g2_sb[:, R:R + 1], W2aug_sb[:],
                             start=True, stop=True)
            cs2_sb = cst.tile([1, NCLASS], f32r)
            nc.vector.tensor_copy(cs2_sb[:], cs2_ps[:])
            nc.vector.tensor_copy(g2_sb[:, 0:512], gps[0][:, :])
            nc.scalar.copy(g2_sb[:, 512:1024], gps[1][:, :])
            nc.vector.tensor_copy(g2_sb[:, 1024:R], gps[2][:, :R - 1024])
            o_tags = ["o_psA", "o_psB", "o_psA"]
            add_engs = [nc.vector, nc.scalar, nc.scalar]  # gpsimd can't read PSUM
            dma_engs = [nc.sync, nc.scalar, nc.sync]
            for q, (c0, cn) in enumerate(R_SPLITS):
                o_ps = ps_e.tile([NCLASS, 512], f32, name=o_tags[q], tag=o_tags[q])
                nc.tensor.matmul(o_ps[:, :cn], W2aug_sb[:], g2_sb[:, c0:c0 + cn],
                                 start=True, stop=False)
                # rank-1 zero-point correction: += cs2 (x) muT (1-partition mm)
                nc.tensor.matmul(o_ps[:, :cn], cs2_sb[:], mu_sb[:, c0:c0 + cn],
                                 start=False, stop=True)
                cnr = min(c0 + cn, R) - c0          # clip padded columns
                if add_engs[q] is nc.scalar:
                    nc.scalar.add(o_sb[:, c0:c0 + cn], o_ps[:, :cn], c_sb[:, 0:1])
                else:
                    add_engs[q].tensor_scalar_add(o_sb[:, c0:c0 + cn],
                                                  o_ps[:, :cn], c_sb[:, 0:1])
                dma_engs[q].dma_start(outT_d[:, c0:c0 + cnr], o_sb[:, c0:c0 + cnr])

    nc.finalize()
    return nc


def _get_nc():
    if "nc" not in _cached:
        _cached["nc"] = _build_nc()
    return _cached["nc"]


def _prep_in_maps(x, adj, gc_W, gc_b, fc_W, fc_b):
    f = np.float32
    x = np.asarray(x, dtype=f)
    adj = np.asarray(adj, dtype=f)

    # ---- quantization scales (shared across cores) ----
    mu = adj.mean(axis=1, dtype=np.float64).astype(f)          # per-row zero point
    dmax = float(np.max(np.abs(adj - mu[:, None])))
    SD = 126.0 / dmax                                          # e4m3 sweet spot
    amax = float(np.abs(x).max())
    Sxh = 2.0 ** np.floor(np.log2(224.0 / max(amax, 1e-30)))
    xhi = (x * f(Sxh)).astype(F8)
    xr = x - xhi.astype(f) / f(Sxh)
    rmax = float(np.abs(xr).max())
    Sxl = 2.0 ** np.floor(np.log2(224.0 / max(rmax, 1e-30)))
    xlo = (xr * f(Sxl)).astype(F8)

    # ---- x pairs: xP[p, t, i, 0:32|32:64] = xhi|xlo row k, k = 256t+128i+p ----
    xpad = np.zeros((NPAD, NH2), dtype=F8)
    xpad[:N, :NHID] = xhi
    xpad[:N, NHID:] = xlo
    xP = np.ascontiguousarray(
        xpad.reshape(NT2, 2, KP, NH2).transpose(2, 0, 1, 3).reshape(KP, -1))

    # ---- per-core D^T blocks with in-slab p-major interleave ----
    adjT = []
    for c in range(NCORES):
        blk = adj[c * R:(c + 1) * R, :]                        # [1500, 12000]
        mu_c = mu[c * R:(c + 1) * R]
        Dq = ((blk - mu_c[:, None]) * f(SD)).astype(F8)        # [1500, 12000]
        DT = np.zeros((NPAD, RP), dtype=F8)
        DT[:N, :R] = Dq.T
        DT[:N, R] = f(1.0)                                     # colsum column
        # A5[t, p, i, :] = DT[256t + 128i + p]
        A5 = DT.reshape(NT2, 2, KP, RP).transpose(0, 2, 1, 3)  # [t, p, i, r]
        rows = np.empty((NT2 * KP, 2, RP), dtype=F8)
        s = 0
        for G in GROUPS:
            chunk = A5[s:s + G].transpose(1, 0, 2, 3)          # [p, j, i, r]
            rows[s * KP:(s + G) * KP] = chunk.reshape(G * KP, 2, RP)
            s += G
        adjT.append(np.ascontiguousarray(rows.reshape(NT2 * KP, 2 * RP)))

    gcWT = np.asarray(gc_W, dtype=f).T
    gcWT2 = np.ascontiguousarray(np.concatenate([gcWT, gcWT], axis=1))
    fcW = np.ascontiguousarray(np.asarray(fc_W, dtype=f))
    fcWb = np.ascontiguousarray(
        np.concatenate([fcW, np.asarray(fc_b, dtype=f).reshape(1, NCLASS)], axis=0))
    gcb1 = np.ascontiguousarray(
        np.concatenate([np.asarray(gc_b, dtype=f).reshape(NHID, 1),
                        np.ones((1, 1), dtype=f)], axis=0))
    gam = np.empty((NH2, 1), dtype=f)
    gam[:NHID, 0] = f(1.0 / (SD * Sxh))
    gam[NHID:, 0] = f(1.0 / (SD * Sxl))
    muTs = []
    for c in range(NCORES):
        m = np.zeros((1, RP), dtype=f)
        m[0, :R] = mu[c * R:(c + 1) * R] * f(SD)
        muTs.append(m)
    return [{"xP": xP, "adjT": adjT[c], "gcWT2": gcWT2, "fcW": fcW,
             "fcWb": fcWb, "gcb1": gcb1, "gam": gam, "muT": muTs[c]}
            for c in range(NCORES)]


def run_traced(x, adj, gc_W, gc_b, fc_W, fc_b, trace=False, **kw):
    """Run on the 8 NeuronCores; returns (out [N, NCLASS] f32, BassKernelResults)."""
    from concourse.bass_utils import run_bass_kernel_spmd

    # NOTE: walrus --enable-ldw-opt=true rejects DoubleRow Ldweights
    # ("InstLdweights is not compatible with LDW optimization"), so unlike the
    # fp32 baseline we leave it off; the DMA-bound main loop has PE slack.
    nc = _get_nc()
    in_maps = _prep_in_maps(x, adj, gc_W, gc_b, fc_W, fc_b)
    res = run_bass_kernel_spmd(nc, in_maps, list(range(NCORES)), trace=trace, **kw)
    outT = np.concatenate([res.results[c]["outT"] for c in range(NCORES)], axis=1)
    out = np.ascontiguousarray(outT.T).astype(np.float32, copy=False)
    return out, res


def kernel(x, adj, gc_W, gc_b, fc_W, fc_b):
    out, _ = run_traced(x, adj, gc_W, gc_b, fc_W, fc_b, trace=False)
    return out


# revision 18
# speedup vs baseline: 1.0250x; 1.0250x over previous
"""Trainium2 Bass kernel for GCN ExitBlock: out = (adj @ (x @ gc_W) + gc_b) @ fc_W + fc_b.

Strategy (8 NeuronCores, SPMD, no collectives):
  - Reassociate: out = ((adj @ x) @ gc_W + gc_b) @ fc_W + fc_b, row-sharding the
    output so core c computes rows [1500c, 1500(c+1)).
  - The kernel is HBM-bound on streaming adj (576 MB fp32).  Quantize it to
    fp8 e4m3 with a per-row zero point: adj[i,:] = mu_i + D[i,:], where D is
    quantized (uniform residual in [-1/N, 1/N] uses the fp8 grid ~2x better
    than the one-sided raw values).  HBM traffic drops 4x -> ~19 MB/core.
  - The rank-1 zero-point term mu_i * colsum(x) @ W2 is reconstructed exactly
    on-device: an all-ones extra moving column makes the main matmul emit
    colsum(x_q) for free, and a 1-partition outer-product matmul adds
    cs2 (x) muT into the output PSUM group.
  - x is split into fp8 (hi, lo) column pairs (64 stationary columns) so x's
    quantization error is second-order; scales are folded into the fused
    classifier weights W2 = gc_W @ fc_W on device via a per-partition gamma.
  - Main loop: DoubleRow fp8 matmuls contract 256 k-rows per pass
    (stationary [128,2,64] = x pairs, moving [128,2,1504] = D^T pairs).
    k pair-tiles are batched into ~1.5 MB slabs on 2 alternating DMA rings.
  - Epilogue fused on-device: outT = W2aug^T @ g + cs2 (x) muT + c per column
    chunk; host gathers the 8 outT blocks ([16, 1500]) into [12000, 16].

Measured-precision note: full-pipeline numpy sim gives rel err 1.24e-2 vs the
fp32 reference (gate 2e-2); the error is dominated by e4m3 quantization of D.
HBM-bound: ~19 MB/core @ ~367 GB/s => ~52 us roofline.
"""
import sys

sys.path.insert(0, "/opt/trn_rl_repo")

import numpy as np
import ml_dtypes

F8 = ml_dtypes.float8_e4m3

N, NHID, NCLASS, NCORES = 12000, 32, 16, 8
R = N // NCORES            # 1500 rows per core
RP = 1504                  # padded moving columns; col 1500 = ones (colsum), rest 0
KP = 128                   # partitions per sub-tile
NT2 = 47                   # pair-tiles (12032 padded k rows / 256)
NPAD = NT2 * 2 * KP        # 12032
NH2 = 2 * NHID             # 64 stationary cols: [x_hi | x_lo]
# pair-tiles per slab (4 => ~1.5 MB).  At fp8 the whole adj working set is
# only ~141 KB/partition, so EVERY slab gets its own dedicated SBUF buffer
# and all slab DMAs are issued upfront: the rings stream back-to-back with
# no buffer backpressure and no inter-slab dead time (a shared 6-buf pool
# measured ~0.5-7 us of ring stalls whenever the in-order PE wait on the
# momentarily-slower ring filled the pool).  Taper the tail so the final
# matmul backlog after the last byte is one tiny slab per ring.
GROUPS = [4] * 10 + [3, 2, 1, 1]
assert sum(GROUPS) == NT2
R_SPLITS = [(0, 512), (512, 512), (1024, RP - 1024)]          # psum bank <= 512

_cached = {}


def _enable_ldw_opt():
    """Flip walrus --enable-ldw-opt to true for this kernel's compile.

    concourse hardcodes it off; enabling lets walrus dedupe the three
    identical back-to-back LDWEIGHTS per pair-tile (same stationary operand
    for the three output-column chunks), removing a PE stall before nearly
    every matmul.
    """
    if _cached.get("ldw_patched"):
        return
    from concourse import bass_utils

    orig = bass_utils.run_command

    def patched(cmd, *a, **kw):
        if isinstance(cmd, list):
            cmd = ["--enable-ldw-opt=true" if c == "--enable-ldw-opt=false" else c
                   for c in cmd]
        return orig(cmd, *a, **kw)

    bass_utils.run_command = patched
    _cached["ldw_patched"] = True


def _build_nc():
    import concourse.bacc as bacc
    import concourse.mybir as mybir
    from concourse import tile

    f32 = mybir.dt.float32
    f32r = mybir.dt.float32r
    f8 = mybir.dt.float8e4
    DR = mybir.MatmulPerfMode.DoubleRow

    nc = bacc.Bacc()
    xP_d = nc.declare_dram_parameter("xP", [KP, NT2 * 2 * NH2], f8, isOutput=False)
    adjT_d = nc.declare_dram_parameter("adjT", [NT2 * KP, 2 * RP], f8, isOutput=False)
    gcWT2_d = nc.declare_dram_parameter("gcWT2", [NHID, NH2], f32, isOutput=False)
    fcW_d = nc.declare_dram_parameter("fcW", [NHID, NCLASS], f32, isOutput=False)
    # cvec = [gc_b; 1] followed by [fc_W; fc_b] rows -> bias vector via matmul
    fcWb_d = nc.declare_dram_parameter("fcWb", [NHID + 1, NCLASS], f32, isOutput=False)
    gcb1_d = nc.declare_dram_parameter("gcb1", [NHID + 1, 1], f32, isOutput=False)
    gam_d = nc.declare_dram_parameter("gam", [NH2, 1], f32, isOutput=False)
    muT_d = nc.declare_dram_parameter("muT", [1, RP], f32r, isOutput=False)
    outT_d = nc.declare_dram_parameter("outT", [NCLASS, R], f32, isOutput=True)

    with tile.TileContext(nc) as tc:
        with (
            tc.tile_pool(name="cst", bufs=1) as cst,
            tc.tile_pool(name="adj", bufs=1) as adjp,
            tc.tile_pool(name="ps_g", bufs=1, space="PSUM") as ps_g,
            tc.tile_pool(name="ps_e", bufs=1, space="PSUM") as ps_e,
        ):
            # ---- constant preloads (small weights on the scalar ring) ----
            x_sb = cst.tile([KP, NT2, 2, NH2], f8)
            gcWT2_sb = cst.tile([NHID, NH2], f32)
            nc.scalar.dma_start(gcWT2_sb[:], gcWT2_d[:])
            fcW_sb = cst.tile([NHID, NCLASS], f32)
            nc.scalar.dma_start(fcW_sb[:], fcW_d[:])
            fcWb_sb = cst.tile([NHID + 1, NCLASS], f32)
            nc.scalar.dma_start(fcWb_sb[:], fcWb_d[:])
            gcb1_sb = cst.tile([NHID + 1, 1], f32)
            nc.scalar.dma_start(gcb1_sb[:], gcb1_d[:])
            gam_sb = cst.tile([NH2, 1], f32)
            nc.scalar.dma_start(gam_sb[:], gam_d[:])
            mu_sb = cst.tile([1, RP], f32r)
            nc.sync.dma_start(mu_sb[:], muT_d[:])

            gps = [ps_g.tile([NH2, n], f32, name=f"gps{j}", tag=f"gps{j}")
                   for j, (_, n) in enumerate(R_SPLITS)]

            # bias vector c = fcWb.T @ [gc_b; 1] = fc_W.T gc_b + fc_b  [16, 1]
            c_ps = ps_e.tile([NCLASS, 1], f32, name="c_ps", tag="c_ps")
            nc.tensor.matmul(c_ps[:], fcWb_sb[:], gcb1_sb[:], start=True, stop=True)
            c_sb = cst.tile([NCLASS, 1], f32)
            nc.vector.tensor_copy(c_sb[:], c_ps[:])
            # stacked fused classifier weights [W2; W2] scaled by gamma  [64, 16]
            w2_ps = ps_e.tile([NH2, NCLASS], f32, name="w2_ps", tag="w2_ps")
            nc.tensor.matmul(w2_ps[:], gcWT2_sb[:], fcW_sb[:], start=True, stop=True)
            W2aug_sb = cst.tile([NH2, NCLASS], f32r)
            nc.vector.tensor_scalar_mul(W2aug_sb[:], w2_ps[:], gam_sb[:, 0:1])

            # ---- main streaming loop: g[64,1504] += xpair.T @(DR) DTpair ----
            # All of x is loaded upfront (one transfer per ring) -- it lives
            # in SBUF for the whole kernel, and per-slab x chunks just added
            # ~0.6 us of ring dead time each.
            xP4 = xP_d.rearrange("p (t i m) -> p t i m", i=2, m=NH2)
            nc.sync.dma_start(x_sb[:, 0:24, :, :], xP4[:, 0:24, :, :])
            nc.scalar.dma_start(x_sb[:, 24:NT2, :, :], xP4[:, 24:NT2, :, :])
            # Issue ALL slab DMAs upfront (dedicated buffer per slab, no
            # reuse): each ring's descriptors queue back-to-back.
            # 2 HWDGE rings (sync/scalar).  A third ring on gpsimd (SWDGE)
            # measured WORSE: all queues sank to ~100 GB/s (aggregate 196 vs
            # 373 GB/s with two HWDGE rings, which already sit at the ~358
            # GB/s per-NC HBM limit).
            slabs = []
            k0 = 0
            for g, G in enumerate(GROUPS):
                eng = nc.sync if (g % 2 == 0) else nc.scalar
                a_sb = adjp.tile([KP, G, 2, RP], f8, name=f"a{g}", tag=f"a{g}")
                eng.dma_start(
                    a_sb[:, :, :, :],
                    adjT_d[k0:k0 + KP * G, :].rearrange(
                        "(p j) (i r) -> p j i r", j=G, i=2))
                slabs.append(a_sb)
                k0 += KP * G
            s = 0          # global pair-tile index
            for g, G in enumerate(GROUPS):
                a_sb = slabs[g]
                for j in range(G):
                    st = (s == 0)
                    sp = (s == NT2 - 1)
                    for q, (c0, cn) in enumerate(R_SPLITS):
                        nc.tensor.matmul(gps[q][:, :cn], x_sb[:, s, :, :],
                                         a_sb[:, j, :, c0:c0 + cn],
                                         start=st, stop=sp, perf_mode=DR)
                    s += 1

            # ---- epilogue: outT = W2aug.T @ g + cs2 (x) muT + c ----
            # Parallelized across engines: the colsum column is copied first
            # (it feeds cs2), the three big PSUM->SBUF copies run on
            # vector/scalar concurrently, each chunk gets its own PSUM bank,
            # and the bias adds alternate vector/scalar.
            g2_sb = cst.tile([NH2, RP], f32r)
            o_sb = cst.tile([NCLASS, RP], f32)
            nc.vector.tensor_copy(g2_sb[:, R:R + 1], gps[2][:, R - 1024:R - 1023])
            # cs2 = colsum_q @ (gamma*W2stack)  [1, 16] on partition 0
            cs2_ps = ps_e.tile([1, NCLASS], f32, name="cs2_ps", tag="cs2_ps")
            nc.tensor.matmul(cs2_ps[:], g2_sb[:, R:R + 1], W2aug_sb[:],
                             start=True, stop=True)
            cs2_sb = cst.tile([1, NCLASS], f32r)
            nc.vector.tensor_copy(cs2_sb[:], cs2_ps[:])
            nc.vector.tensor_copy(g2_sb[:, 0:512], gps[0][:, :])
            nc.scalar.copy(g2_sb[:, 512:1024], gps[1][:, :])
            nc.vector.tensor_copy(g2_sb[:, 1024:R], gps[2][:, :R - 1024])
            o_tags = ["o_psA", "o_psB", "o_psA"]
            add_engs = [nc.vector, nc.scalar, nc.scalar]  # gpsimd can't read PSUM
            dma_engs = [nc.sync, nc.scalar, nc.sync]
            for q, (c0, cn) in enumerate(R_SPLITS):
                o_ps = ps_e.tile([NCLASS, 512], f32, name=o_tags[q], tag=o_tags[q])
                nc.tensor.matmul(o_ps[:, :cn], W2aug_sb[:], g2_sb[:, c0:c0 + cn],
                                 start=True, stop=False)
                # rank-1 zero-point correction: += cs2 (x) muT (1-partition mm)
                nc.tensor.matmul(o_ps[:, :cn], cs2_sb[:], mu_sb[:, c0:c0 + cn],
                                 start=False, stop=True)
                cnr = min(c0 + cn, R) - c0          # clip padded columns
                if add_engs[q] is nc.scalar:
                    nc.scalar.add(o_sb[:, c0:c0 + cn], o_ps[:, :cn], c_sb[:, 0:1])
                else:
                    add_engs[q].tensor_scalar_add(o_sb[:, c0:c0 + cn],
                                                  o_ps[:, :cn], c_sb[:, 0:1])
                dma_engs[q].dma_start(outT_d[:, c0:c0 + cnr], o_sb[:, c0:c0 + cnr])

    nc.finalize()
    return nc


def _get_nc():
    if "nc" not in _cached:
        _cached["nc"] = _build_nc()
    return _cached["nc"]


def _prep_in_maps(x, adj, gc_W, gc_b, fc_W, fc_b):
    f = np.float32
    x = np.asarray(x, dtype=f)
    adj = np.asarray(adj, dtype=f)

    # ---- quantization scales (shared across cores) ----
    mu = adj.mean(axis=1, dtype=np.float64).astype(f)          # per-row zero point
    dmax = float(np.max(np.abs(adj - mu[:, None])))
    SD = 126.0 / dmax                                          # e4m3 sweet spot
    amax = float(np.abs(x).max())
    Sxh = 2.0 ** np.floor(np.log2(224.0 / max(amax, 1e-30)))
    xhi = (x * f(Sxh)).astype(F8)
    xr = x - xhi.astype(f) / f(Sxh)
    rmax = float(np.abs(xr).max())
    Sxl = 2.0 ** np.floor(np.log2(224.0 / max(rmax, 1e-30)))
    xlo = (xr * f(Sxl)).astype(F8)

    # ---- x pairs: xP[p, t, i, 0:32|32:64] = xhi|xlo row k, k = 256t+128i+p ----
    xpad = np.zeros((NPAD, NH2), dtype=F8)
    xpad[:N, :NHID] = xhi
    xpad[:N, NHID:] = xlo
    xP = np.ascontiguousarray(
        xpad.reshape(NT2, 2, KP, NH2).transpose(2, 0, 1, 3).reshape(KP, -1))

    # ---- per-core D^T blocks with in-slab p-major interleave ----
    adjT = []
    for c in range(NCORES):
        blk = adj[c * R:(c + 1) * R, :]                        # [1500, 12000]
        mu_c = mu[c * R:(c + 1) * R]
        Dq = ((blk - mu_c[:, None]) * f(SD)).astype(F8)        # [1500, 12000]
        DT = np.zeros((NPAD, RP), dtype=F8)
        DT[:N, :R] = Dq.T
        DT[:N, R] = f(1.0)                                     # colsum column
        # A5[t, p, i, :] = DT[256t + 128i + p]
        A5 = DT.reshape(NT2, 2, KP, RP).transpose(0, 2, 1, 3)  # [t, p, i, r]
        rows = np.empty((NT2 * KP, 2, RP), dtype=F8)
        s = 0
        for G in GROUPS:
            chunk = A5[s:s + G].transpose(1, 0, 2, 3)          # [p, j, i, r]
            rows[s * KP:(s + G) * KP] = chunk.reshape(G * KP, 2, RP)
            s += G
        adjT.append(np.ascontiguousarray(rows.reshape(NT2 * KP, 2 * RP)))

    gcWT = np.asarray(gc_W, dtype=f).T
    gcWT2 = np.ascontiguousarray(np.concatenate([gcWT, gcWT], axis=1))
    fcW = np.ascontiguousarray(np.asarray(fc_W, dtype=f))
    fcWb = np.ascontiguousarray(
        np.concatenate([fcW, np.asarray(fc_b, dtype=f).reshape(1, NCLASS)], axis=0))
    gcb1 = np.ascontiguousarray(
        np.concatenate([np.asarray(gc_b, dtype=f).reshape(NHID, 1),
                        np.ones((1, 1), dtype=f)], axis=0))
    gam = np.empty((NH2, 1), dtype=f)
    gam[:NHID, 0] = f(1.0 / (SD * Sxh))
    gam[NHID:, 0] = f(1.0 / (SD * Sxl))
    muTs = []
    for c in range(NCORES):
        m = np.zeros((1, RP), dtype=f)
        m[0, :R] = mu[c * R:(c + 1) * R] * f(SD)
        muTs.append(m)
    return [{"xP": xP, "adjT": adjT[c], "gcWT2": gcWT2, "fcW": fcW,
             "fcWb": fcWb, "gcb1": gcb1, "gam": gam, "muT": muTs[c]}
            for c in range(NCORES)]


def run_traced(x, adj, gc_W, gc_b, fc_W, fc_b, trace=False, **kw):
    """Run on the 8 NeuronCores; returns (out [N, NCLASS] f32, BassKernelResults)."""
    from concourse.bass_utils import run_bass_kernel_spmd

    # NOTE: walrus --enable-ldw-opt=true rejects DoubleRow Ldweights
    # ("InstLdweights is not compatible with LDW optimization"), so unlike the
    # fp32 baseline we leave it off; the DMA-bound main loop has PE slack.
    nc = _get_nc()
    in_maps = _prep_in_maps(x, adj, gc_W, gc_b, fc_W, fc_b)
    res = run_bass_kernel_spmd(nc, in_maps, list(range(NCORES)), trace=trace, **kw)
    outT = np.concatenate([res.results[c]["outT"] for c in range(NCORES)], axis=1)
    out = np.ascontiguousarray(outT.T).astype(np.float32, copy=False)
    return out, res


def kernel(x, adj, gc_W, gc_b, fc_W, fc_b):
    out, _ = run_traced(x, adj, gc_W, gc_b, fc_W, fc_b, trace=False)
    return out


# revision 23
# speedup vs baseline: 1.1695x; 1.1410x over previous
"""Trainium2 Bass kernel for GCN ExitBlock: out = (adj @ (x @ gc_W) + gc_b) @ fc_W + fc_b.

Strategy (8 NeuronCores, SPMD, no collectives):
  - Reassociate: out = ((adj @ x) @ gc_W + gc_b) @ fc_W + fc_b, row-sharding the
    output so core c computes rows [1500c, 1500(c+1)).
  - The kernel is HBM-bound on streaming adj (576 MB fp32).  Quantize it to
    fp8 e4m3 with a per-row zero point: adj[i,:] = mu_i + D[i,:], where D is
    quantized (uniform residual in [-1/N, 1/N] uses the fp8 grid ~2x better
    than the one-sided raw values).  HBM traffic drops 4x -> ~19 MB/core.
  - The rank-1 zero-point term mu_i * colsum(x) @ W2 is reconstructed exactly
    on-device: an all-ones extra moving column makes the main matmul emit
    colsum(x_q) for free, and a 1-partition outer-product matmul adds
    cs2 (x) muT into the output PSUM group.
  - x is split into fp8 (hi, lo) column pairs (64 stationary columns) so x's
    quantization error is second-order; scales are folded into the fused
    classifier weights W2 = gc_W @ fc_W on device via a per-partition gamma.
  - Main loop: DoubleRow fp8 matmuls contract 256 k-rows per pass
    (stationary [128,2,64] = x pairs, moving [128,2,1504] = D^T pairs).
    k pair-tiles are batched into ~1.5 MB slabs on 2 alternating DMA rings.
  - Epilogue fused on-device: outT = W2aug^T @ g + cs2 (x) muT + c per column
    chunk; host gathers the 8 outT blocks ([16, 1500]) into [12000, 16].

Measured-precision note: full-pipeline numpy sim gives rel err 1.24e-2 vs the
fp32 reference (gate 2e-2); the error is dominated by e4m3 quantization of D.
HBM-bound: ~19 MB/core @ ~367 GB/s => ~52 us roofline.
"""
import sys

sys.path.insert(0, "/opt/trn_rl_repo")

import numpy as np
import ml_dtypes

F8 = ml_dtypes.float8_e4m3

N, NHID, NCLASS, NCORES = 12000, 32, 16, 8
R = N // NCORES            # 1500 rows per core
RP = 1504                  # padded moving columns; col 1500 = ones (colsum), rest 0
KP = 128                   # partitions per sub-tile
NT2 = 47                   # pair-tiles (12032 padded k rows / 256)
NPAD = NT2 * 2 * KP        # 12032
NH2 = 2 * NHID             # 64 stationary cols: [x_hi | x_lo]
# pair-tiles per slab (4 => ~1.5 MB).  At fp8 the whole adj working set is
# only ~141 KB/partition, so EVERY slab gets its own dedicated SBUF buffer
# and all slab DMAs are issued upfront: the rings stream back-to-back with
# no buffer backpressure and no inter-slab dead time (a shared 6-buf pool
# measured ~0.5-7 us of ring stalls whenever the in-order PE wait on the
# momentarily-slower ring filled the pool).  Taper the tail so the final
# matmul backlog after the last byte is one tiny slab per ring.
GROUPS = [4] * 8 + [3, 3, 2, 2, 1, 1, 1, 1, 1]
assert sum(GROUPS) == NT2
R_SPLITS = [(0, 512), (512, 512), (1024, RP - 1024)]          # psum bank <= 512

_cached = {}


def _enable_ldw_opt():
    """Flip walrus --enable-ldw-opt to true for this kernel's compile.

    concourse hardcodes it off; enabling lets walrus dedupe the three
    identical back-to-back LDWEIGHTS per pair-tile (same stationary operand
    for the three output-column chunks), removing a PE stall before nearly
    every matmul.
    """
    if _cached.get("ldw_patched"):
        return
    from concourse import bass_utils

    orig = bass_utils.run_command

    def patched(cmd, *a, **kw):
        if isinstance(cmd, list):
            cmd = ["--enable-ldw-opt=true" if c == "--enable-ldw-opt=false" else c
                   for c in cmd]
        return orig(cmd, *a, **kw)

    bass_utils.run_command = patched
    _cached["ldw_patched"] = True


def _build_nc():
    import concourse.bacc as bacc
    import concourse.mybir as mybir
    from concourse import tile

    f32 = mybir.dt.float32
    f32r = mybir.dt.float32r
    f8 = mybir.dt.float8e4
    DR = mybir.MatmulPerfMode.DoubleRow

    nc = bacc.Bacc()
    xP_d = nc.declare_dram_parameter("xP", [KP, NT2 * 2 * NH2], f8, isOutput=False)
    adjT_d = nc.declare_dram_parameter("adjT", [NT2 * KP, 2 * RP], f8, isOutput=False)
    # all small weights packed into ONE tensor (one DMA issue):
    #   cols 0:64 = [gc_W.T | gc_W.T] (rows 0:32), cols 64:80 = [fc_W; fc_b]
    #   (rows 0:33), col 80 = [gc_b; 1] (rows 0:33), col 81 = gamma (rows 0:64)
    cw_d = nc.declare_dram_parameter("cw", [NH2, 82], f32, isOutput=False)
    muT_d = nc.declare_dram_parameter("muT", [1, RP], f32r, isOutput=False)
    outT_d = nc.declare_dram_parameter("outT", [NCLASS, R], f32, isOutput=True)

    with tile.TileContext(nc) as tc:
        with (
            tc.tile_pool(name="cst", bufs=1) as cst,
            tc.tile_pool(name="adj", bufs=1) as adjp,
            tc.tile_pool(name="ps_g", bufs=1, space="PSUM") as ps_g,
            tc.tile_pool(name="ps_e", bufs=1, space="PSUM") as ps_e,
        ):
            # ---- constant preloads (one packed DMA per ring) ----
            x_sb = cst.tile([KP, NT2, 2, NH2], f8)
            cw_sb = cst.tile([NH2, 82], f32)
            nc.scalar.dma_start(cw_sb[:], cw_d[:])
            gcWT2_sb = cw_sb[0:NHID, 0:NH2]
            fcW_sb = cw_sb[0:NHID, NH2:NH2 + NCLASS]
            fcWb_sb = cw_sb[0:NHID + 1, NH2:NH2 + NCLASS]
            gcb1_sb = cw_sb[0:NHID + 1, 80:81]
            gam_sb = cw_sb[0:NH2, 81:82]
            mu_sb = cst.tile([1, RP], f32r)
            nc.sync.dma_start(mu_sb[:], muT_d[:])

            # single 3-bank PSUM accumulator [64, 1536]; chunk q's matmuls
            # write the bank-aligned slice [:, 512q : 512q+cn]
            gbig = ps_g.tile([NH2, 1536], f32, name="gbig", tag="gbig")
            gps = [gbig[:, 512 * q:512 * q + n]
                   for q, (_, n) in enumerate(R_SPLITS)]

            # bias vector c = fcWb.T @ [gc_b; 1] = fc_W.T gc_b + fc_b  [16, 1]
            c_ps = ps_e.tile([NCLASS, 1], f32, name="c_ps", tag="c_ps")
            nc.tensor.matmul(c_ps[:], fcWb_sb, gcb1_sb, start=True, stop=True)
            c_sb = cst.tile([NCLASS, 1], f32)
            nc.vector.tensor_copy(c_sb[:], c_ps[:])
            # stacked fused classifier weights [W2; W2] scaled by gamma  [64, 16]
            w2_ps = ps_e.tile([NH2, NCLASS], f32, name="w2_ps", tag="w2_ps")
            nc.tensor.matmul(w2_ps[:], gcWT2_sb, fcW_sb, start=True, stop=True)
            W2aug_sb = cst.tile([NH2, NCLASS], f32r)
            nc.vector.tensor_scalar_mul(W2aug_sb[:], w2_ps[:], gam_sb)

            # ---- main streaming loop: g[64,1504] += xpair.T @(DR) DTpair ----
            # All of x is loaded upfront (one transfer per ring) -- it lives
            # in SBUF for the whole kernel, and per-slab x chunks just added
            # ~0.6 us of ring dead time each.
            xP4 = xP_d.rearrange("p (t i m) -> p t i m", i=2, m=NH2)
            nc.sync.dma_start(x_sb[:, 0:24, :, :], xP4[:, 0:24, :, :])
            nc.scalar.dma_start(x_sb[:, 24:NT2, :, :], xP4[:, 24:NT2, :, :])
            # Issue ALL slab DMAs upfront (dedicated buffer per slab, no
            # reuse): each ring's descriptors queue back-to-back.
            # 2 HWDGE rings (sync/scalar).  A third ring on gpsimd (SWDGE)
            # measured WORSE: all queues sank to ~100 GB/s (aggregate 196 vs
            # 373 GB/s with two HWDGE rings, which already sit at the ~358
            # GB/s per-NC HBM limit).
            slabs = []
            k0 = 0
            for g, G in enumerate(GROUPS):
                eng = nc.sync if (g % 2 == 0) else nc.scalar
                a_sb = adjp.tile([KP, G, 2, RP], f8, name=f"a{g}", tag=f"a{g}")
                eng.dma_start(
                    a_sb[:, :, :, :],
                    adjT_d[k0:k0 + KP * G, :].rearrange(
                        "(p j) (i r) -> p j i r", j=G, i=2))
                slabs.append(a_sb)
                k0 += KP * G
            s = 0          # global pair-tile index
            for g, G in enumerate(GROUPS):
                a_sb = slabs[g]
                for j in range(G):
                    st = (s == 0)
                    sp = (s == NT2 - 1)
                    for q, (c0, cn) in enumerate(R_SPLITS):
                        nc.tensor.matmul(gps[q][:, :cn], x_sb[:, s, :, :],
                                         a_sb[:, j, :, c0:c0 + cn],
                                         start=st, stop=sp, perf_mode=DR)
                    s += 1

            # ---- epilogue: outT = W2aug.T @ g + cs2 (x) muT + c ----
            # Parallelized across engines: the colsum column is copied first
            # (it feeds cs2), the three big PSUM->SBUF copies run on
            # vector/scalar concurrently, each chunk gets its own PSUM bank,
            # and the bias adds alternate vector/scalar.
            g2_sb = cst.tile([NH2, RP], f32r)
            o_sb = cst.tile([NCLASS, RP], f32)
            # one bank-spanning PSUM->SBUF copy (DVE fixed cost ~0.6 us/op
            # dominated the old 3-copy epilogue)
            nc.vector.tensor_copy(g2_sb[:, 0:RP], gbig[:, 0:RP])
            # cs2 = colsum_q @ (gamma*W2stack)  [1, 16] on partition 0
            cs2_ps = ps_e.tile([1, NCLASS], f32, name="cs2_ps", tag="cs2_ps")
            nc.tensor.matmul(cs2_ps[:], g2_sb[:, R:R + 1], W2aug_sb[:],
                             start=True, stop=True)
            cs2_sb = cst.tile([1, NCLASS], f32r)
            nc.vector.tensor_copy(cs2_sb[:], cs2_ps[:])
            o_tags = ["o_psA", "o_psB", "o_psA"]
            add_engs = [nc.vector, nc.scalar, nc.scalar]  # gpsimd can't read PSUM
            dma_engs = [nc.sync, nc.scalar, nc.sync]
            for q, (c0, cn) in enumerate(R_SPLITS):
                o_ps = ps_e.tile([NCLASS, 512], f32, name=o_tags[q], tag=o_tags[q])
                nc.tensor.matmul(o_ps[:, :cn], W2aug_sb[:], g2_sb[:, c0:c0 + cn],
                                 start=True, stop=False)
                # rank-1 zero-point correction: += cs2 (x) muT (1-partition mm)
                nc.tensor.matmul(o_ps[:, :cn], cs2_sb[:], mu_sb[:, c0:c0 + cn],
                                 start=False, stop=True)
                cnr = min(c0 + cn, R) - c0          # clip padded columns
                if add_engs[q] is nc.scalar:
                    nc.scalar.add(o_sb[:, c0:c0 + cn], o_ps[:, :cn], c_sb[:, 0:1])
                else:
                    add_engs[q].tensor_scalar_add(o_sb[:, c0:c0 + cn],
                                                  o_ps[:, :cn], c_sb[:, 0:1])
                dma_engs[q].dma_start(outT_d[:, c0:c0 + cnr], o_sb[:, c0:c0 + cnr])

    nc.finalize()
    return nc


def _get_nc():
    if "nc" not in _cached:
        _cached["nc"] = _build_nc()
    return _cached["nc"]


def _prep_in_maps(x, adj, gc_W, gc_b, fc_W, fc_b):
    f = np.float32
    x = np.asarray(x, dtype=f)
    adj = np.asarray(adj, dtype=f)

    # ---- quantization scales (shared across cores) ----
    mu = adj.mean(axis=1, dtype=np.float64).astype(f)          # per-row zero point
    dmax = float(np.max(np.abs(adj - mu[:, None])))
    SD = 126.0 / dmax                                          # e4m3 sweet spot
    amax = float(np.abs(x).max())
    Sxh = 2.0 ** np.floor(np.log2(224.0 / max(amax, 1e-30)))
    xhi = (x * f(Sxh)).astype(F8)
    xr = x - xhi.astype(f) / f(Sxh)
    rmax = float(np.abs(xr).max())
    Sxl = 2.0 ** np.floor(np.log2(224.0 / max(rmax, 1e-30)))
    xlo = (xr * f(Sxl)).astype(F8)

    # ---- x pairs: xP[p, t, i, 0:32|32:64] = xhi|xlo row k, k = 256t+128i+p ----
    xpad = np.zeros((NPAD, NH2), dtype=F8)
    xpad[:N, :NHID] = xhi
    xpad[:N, NHID:] = xlo
    xP = np.ascontiguousarray(
        xpad.reshape(NT2, 2, KP, NH2).transpose(2, 0, 1, 3).reshape(KP, -1))

    # ---- per-core D^T blocks with in-slab p-major interleave ----
    adjT = []
    for c in range(NCORES):
        blk = adj[c * R:(c + 1) * R, :]                        # [1500, 12000]
        mu_c = mu[c * R:(c + 1) * R]
        Dq = ((blk - mu_c[:, None]) * f(SD)).astype(F8)        # [1500, 12000]
        DT = np.zeros((NPAD, RP), dtype=F8)
        DT[:N, :R] = Dq.T
        DT[:N, R] = f(1.0)                                     # colsum column
        # A5[t, p, i, :] = DT[256t + 128i + p]
        A5 = DT.reshape(NT2, 2, KP, RP).transpose(0, 2, 1, 3)  # [t, p, i, r]
        rows = np.empty((NT2 * KP, 2, RP), dtype=F8)
        s = 0
        for G in GROUPS:
            chunk = A5[s:s + G].transpose(1, 0, 2, 3)          # [p, j, i, r]
            rows[s * KP:(s + G) * KP] = chunk.reshape(G * KP, 2, RP)
            s += G
        adjT.append(np.ascontiguousarray(rows.reshape(NT2 * KP, 2 * RP)))

    gcWT = np.asarray(gc_W, dtype=f).T
    cw = np.zeros((NH2, 82), dtype=f)
    cw[:NHID, :NHID] = gcWT
    cw[:NHID, NHID:NH2] = gcWT
    cw[:NHID, NH2:NH2 + NCLASS] = np.asarray(fc_W, dtype=f)
    cw[NHID, NH2:NH2 + NCLASS] = np.asarray(fc_b, dtype=f)
    cw[:NHID, 80] = np.asarray(gc_b, dtype=f)
    cw[NHID, 80] = 1.0
    cw[:NHID, 81] = f(1.0 / (SD * Sxh))
    cw[NHID:, 81] = f(1.0 / (SD * Sxl))
    muTs = []
    for c in range(NCORES):
        m = np.zeros((1, RP), dtype=f)
        m[0, :R] = mu[c * R:(c + 1) * R] * f(SD)
        muTs.append(m)
    return [{"xP": xP, "adjT": adjT[c], "cw": cw, "muT": muTs[c]}
            for c in range(NCORES)]


def run_traced(x, adj, gc_W, gc_b, fc_W, fc_b, trace=False, **kw):
    """Run on the 8 NeuronCores; returns (out [N, NCLASS] f32, BassKernelResults)."""
    from concourse.bass_utils import run_bass_kernel_spmd

    # NOTE: walrus --enable-ldw-opt=true rejects DoubleRow Ldweights
    # ("InstLdweights is not compatible with LDW optimization"), so unlike the
    # fp32 baseline we leave it off; the DMA-bound main loop has PE slack.
    nc = _get_nc()
    in_maps = _prep_in_maps(x, adj, gc_W, gc_b, fc_W, fc_b)
    res = run_bass_kernel_spmd(nc, in_maps, list(range(NCORES)), trace=trace, **kw)
    outT = np.concatenate([res.results[c]["outT"] for c in range(NCORES)], axis=1)
    out = np.ascontiguousarray(outT.T).astype(np.float32, copy=False)
    return out, res


def kernel(x, adj, gc_W, gc_b, fc_W, fc_b):
    out, _ = run_traced(x, adj, gc_W, gc_b, fc_W, fc_b, trace=False)
    return out
